# revision 2
# baseline (speedup 1.0000x reference)
"""Trainium2 Bass kernel for nn_MHA_65429531787938.

MHA with a faithful-quirk softmax over dim=0 (the batch axis, B=2).
For B=2 the batch-softmax collapses to an elementwise sigmoid:
    attn0 = sigmoid((s0 - s1)/SCALE),  attn1 = 1 - attn0
and (1-A0) @ V1 = colsum(V1) - A0 @ V1, so a single attention matrix
serves both batches.

Sharding: tensor-parallel over the 16 heads -> 2 heads per core
(columns of w_q/w_k/w_v, rows of W_o). Each core consumes the full x
and produces a partial output (its heads' contribution to out = vals @ W_o);
the host sums the 8 partials.

Per-core pipeline (heads h0=2i, h1=2i+1 -> a 128-wide slice of q/k/v dims):
  phase 1: x -> xT (PE transpose); qT,kT,vT projections (N=512 matmuls);
           qT/kT stored batch-stacked per head ([Q0;-Q1] / [K0;K1]);
           vT -> V natural via PE transpose (V1 stored negated), fp16.
  phase 2: d^T = K0@Q0^T - K1@Q1^T in one fused matmul (contraction=128);
           A0^T = sigmoid(d^T/SCALE) on ACT (fp16);
           psum_av = [V0 | -V1]^T-style matmul + rank-1 colsum correction.
  phase 3: out_partial = vals @ W_o_slice (fp16 operands, fp32 psum).

Precision: QK path in float32r (fp32 rounded to 12 mantissa bits, full PE
speed); attention/output tail in fp16. End-to-end rel err ~1.5e-3.
Set MHA_FP32=1 in the environment for a full-fp32 QK path (~3.5e-4).
"""

import os
import numpy as np

import concourse.bacc as bacc
import concourse.mybir as mybir
import concourse.tile as tile
from concourse import bass_utils
from concourse.masks import make_identity

B, S, D, H = 2, 2048, 1024, 16
HD = 64
SCALE = float(D) ** 0.5
NCORES = 8
HPC = H // NCORES            # heads per core = 2
MS = HPC * HD                # per-core slice width = 128
P = 128
NCH = 8                      # phase-1 chunks (B * S/512)
DT16 = mybir.dt.float16
F32 = mybir.dt.float32

USE_F32R = os.environ.get("MHA_FP32", "0") != "1"
PDT = mybir.dt.float32r if USE_F32R else F32   # proj + scores operand dtype


def build():
    nc = bacc.Bacc("TRN2", target_bir_lowering=False, debug=False)

    x_d = nc.dram_tensor("x", [B, S, D], F32, kind="ExternalInput").ap()
    wq_d = nc.dram_tensor("wq", [D, MS], F32, kind="ExternalInput").ap()
    wk_d = nc.dram_tensor("wk", [D, MS], F32, kind="ExternalInput").ap()
    wv_d = nc.dram_tensor("wv", [D, MS], F32, kind="ExternalInput").ap()
    wo_d = nc.dram_tensor("wo", [MS, D], F32, kind="ExternalInput").ap()
    out_d = nc.dram_tensor("out", [B, S, D], F32, kind="ExternalOutput").ap()

    with tile.TileContext(nc) as tc:
        with tc.tile_pool(name="persist", bufs=1) as pp:
            ident32 = pp.tile([P, P], F32)
            make_identity(nc, ident32[:])
            ident16 = pp.tile([P, P], DT16)
            nc.vector.tensor_copy(ident16[:], ident32[:])
            ones512 = pp.tile([1, 512], DT16)
            nc.vector.memset(ones512[:], 1.0)
            ones128 = pp.tile([P, 1], DT16)
            nc.vector.memset(ones128[:], 1.0)

            # weights
            w_sb = {}
            for name, dram in (("wq", wq_d), ("wk", wk_d), ("wv", wv_d)):
                stage = pp.tile([P, D // P, MS], F32, name=f"{name}_stage")
                nc.sync.dma_start(stage[:], dram.rearrange("(t p) m -> p t m", p=P))
                if PDT != F32:
                    cast = pp.tile([P, D // P, MS], PDT, name=f"{name}_sb")
                    nc.vector.tensor_copy(cast[:], stage[:])
                    w_sb[name] = cast
                else:
                    w_sb[name] = stage
            wo_sb = pp.tile([P, 2, 512], DT16)
            nc.gpsimd.dma_start(wo_sb[:], wo_d.rearrange("p (c n) -> p c n", c=2))

            # big persistent tensors
            qsb = pp.tile([P, HPC, S], PDT)      # [(b,hd), head, qpos], b1 negated
            ksb = pp.tile([P, HPC, S], PDT)      # [(b,hd), head, kpos]
            vt_sb = pp.tile([P, B, S], DT16)     # [(h,hd), batch, kpos], b1 negated
            v_sb = pp.tile([P, S // P, HPC, B, HD], DT16)  # [k, ktile, h, b, hd]
            vals_sb = pp.tile([P, B, S], DT16)   # [(h,hd), batch, qpos]
            c1_sb = pp.tile([1, HPC, HD], DT16)  # +colsum(V1) per head

            # ---------------- phase 1: projections ----------------
            with tc.tile_pool(name="p1sb", bufs=3) as p1sb, \
                 tc.tile_pool(name="p1xt", bufs=2) as p1xt, \
                 tc.tile_pool(name="ps1", bufs=3, space="PSUM") as ps1, \
                 tc.tile_pool(name="ps1v", bufs=2, space="PSUM") as ps1v:
                for c in range(NCH):
                    b, j = divmod(c, NCH // B)
                    xt = p1xt.tile([P, D // P, 512], PDT, tag="xt")
                    for blk in range(4):
                        xb = p1sb.tile([P, D], F32, tag="xb")
                        nc.sync.dma_start(
                            xb[:], x_d[b, j * 512 + blk * P: j * 512 + (blk + 1) * P, :]
                        )
                        for g in range(2):
                            pt = ps1.tile([P, 512], F32, tag="tpose")
                            for t4 in range(4):
                                dt_i = g * 4 + t4
                                nc.tensor.transpose(
                                    pt[:, t4 * P:(t4 + 1) * P],
                                    xb[:, dt_i * P:(dt_i + 1) * P],
                                    ident32[:],
                                )
                            nc.vector.tensor_copy(
                                xt[:, g * 4:(g + 1) * 4, blk * P:(blk + 1) * P],
                                pt[:].rearrange("p (t n) -> p t n", t=4),
                            )
                    # q/k projections -> [dims, 512] psum
                    for name, dest, neg in (("wq", qsb, True), ("wk", ksb, False)):
                        ps = ps1.tile([P, 512], F32, tag="proj")
                        for t in range(D // P):
                            nc.tensor.matmul(
                                ps[:], w_sb[name][:, t, :], xt[:, t, :],
                                start=(t == 0), stop=(t == D // P - 1),
                            )
                        for h in range(HPC):
                            sc = -1.0 if (neg and b == 1) else 1.0
                            nc.any.tensor_scalar_mul(
                                dest[b * HD:(b + 1) * HD, h, j * 512:(j + 1) * 512],
                                ps[h * HD:(h + 1) * HD, :],
                                sc,
                            )
                    # vT projection
                    ps = ps1.tile([P, 512], F32, tag="proj")
                    for t in range(D // P):
                        nc.tensor.matmul(
                            ps[:], w_sb["wv"][:, t, :], xt[:, t, :],
                            start=(t == 0), stop=(t == D // P - 1),
                        )
                    nc.any.tensor_scalar_mul(
                        vt_sb[:, b, j * 512:(j + 1) * 512], ps[:],
                        -1.0 if b == 1 else 1.0,
                    )
                # V natural layout via PE transpose (fp16)
                for b in range(B):
                    for t in range(S // P):
                        pvt = ps1v.tile([P, P], DT16, tag="vt")
                        nc.tensor.transpose(
                            pvt[:], vt_sb[:, b, t * P:(t + 1) * P], ident16[:]
                        )
                        nc.vector.tensor_copy(
                            v_sb[:, t, :, b, :],
                            pvt[:].rearrange("p (h d) -> p h d", h=HPC),
                        )

            # ---------------- phase 2: attention ----------------
            with tc.tile_pool(name="p2a", bufs=3) as p2a, \
                 tc.tile_pool(name="ps2d", bufs=2, space="PSUM") as ps2d, \
                 tc.tile_pool(name="ps2av", bufs=2, space="PSUM") as ps2av:
                # colsums of all (h, b) V columns; extract b=1 (stored negated)
                pc1 = ps2av.tile([1, HPC * B * HD], F32, tag="c1")
                for t in range(S // P):
                    nc.tensor.matmul(
                        pc1[:], ones128[:],
                        v_sb[:, t, :, :, :].rearrange("p h b d -> p (h b d)"),
                        start=(t == 0), stop=(t == S // P - 1),
                    )
                nc.any.tensor_scalar_mul(
                    c1_sb[:],
                    pc1[:].rearrange("p (h b d) -> p h b d", h=HPC, b=B)[:, :, 1, :],
                    -1.0,
                )

                for h in range(HPC):
                    for qc in range(S // 512):
                        pav = ps2av.tile([P, 512], F32, tag="av")
                        for tp in range(S // P // 2):
                            pd = ps2d.tile([P, 1024], F32, tag="d")
                            for u in range(2):
                                t = tp * 2 + u
                                nc.tensor.matmul(
                                    pd[:, u * 512:(u + 1) * 512],
                                    ksb[:, h, t * P:(t + 1) * P],
                                    qsb[:, h, qc * 512:(qc + 1) * 512],
                                    start=True, stop=True,
                                )
                            at = p2a.tile([P, 1024], DT16, tag="at")
                            nc.scalar.activation(
                                at[:], pd[:],
                                mybir.ActivationFunctionType.Sigmoid,
                                scale=1.0 / SCALE,
                            )
                            for u in range(2):
                                t = tp * 2 + u
                                nc.tensor.matmul(
                                    pav[:],
                                    v_sb[:, t, h, :, :].rearrange("p b d -> p (b d)"),
                                    at[:, u * 512:(u + 1) * 512],
                                    start=(t == 0), stop=False,
                                )
                        nc.tensor.matmul(
                            pav[HD:2 * HD, :], c1_sb[:, h, :], ones512[:],
                            start=False, stop=True,
                        )
                        for b in range(B):
                            nc.any.tensor_copy(
                                vals_sb[h * HD:(h + 1) * HD, b,
                                        qc * 512:(qc + 1) * 512],
                                pav[b * HD:(b + 1) * HD, :],
                            )

            # ---------------- phase 3: output projection ----------------
            with tc.tile_pool(name="p3o", bufs=3) as p3o, \
                 tc.tile_pool(name="ps3", bufs=3, space="PSUM") as ps3:
                for b in range(B):
                    for si in range(S // P):
                        ot = p3o.tile([P, D], F32, tag="ot")
                        for nch in range(2):
                            po = ps3.tile([P, 512], F32, tag="o")
                            nc.tensor.matmul(
                                po[:],
                                vals_sb[:, b, si * P:(si + 1) * P],
                                wo_sb[:, nch, :],
                                start=True, stop=True,
                            )
                            nc.any.tensor_copy(
                                ot[:, nch * 512:(nch + 1) * 512], po[:]
                            )
                        nc.sync.dma_start(
                            out_d[b, si * P:(si + 1) * P, :], ot[:]
                        )

    nc.compile()
    return nc


_NC = None


def _get_nc():
    global _NC
    if _NC is None:
        _NC = build()
    return _NC


def kernel(x, w_q, w_k, w_v, W_o, _trace=False):
    x = np.ascontiguousarray(np.asarray(x, dtype=np.float32))
    w_q = np.asarray(w_q, dtype=np.float32)
    w_k = np.asarray(w_k, dtype=np.float32)
    w_v = np.asarray(w_v, dtype=np.float32)
    W_o = np.asarray(W_o, dtype=np.float32)

    nc = _get_nc()
    in_maps = []
    for i in range(NCORES):
        cs = slice(i * MS, (i + 1) * MS)
        in_maps.append({
            "x": x,
            "wq": np.ascontiguousarray(w_q[:, cs]),
            "wk": np.ascontiguousarray(w_k[:, cs]),
            "wv": np.ascontiguousarray(w_v[:, cs]),
            "wo": np.ascontiguousarray(W_o[cs, :]),
        })
    res = bass_utils.run_bass_kernel_spmd(
        nc, in_maps, core_ids=list(range(NCORES)), trace=_trace
    )
    out = res.results[0]["out"].astype(np.float32).copy()
    for i in range(1, NCORES):
        out += res.results[i]["out"]
    if _trace:
        return out, res
    return out
